# revision 7
# baseline (speedup 1.0000x reference)
"""Memory-efficient multi-head attention on 8 TRN2 NeuronCores.

Problem (hardcoded): B=2, L=2048, D=1024, H=16, HD=64.
  out = softmax((X_q Wq + bq)(X_k Wk + bk)^T / sqrt(HD)) (X_v Wv + bv) Wo + bo

Sharding: 8 cores = 2 batches x 4 head-groups (4 heads each).  Each core gets
its batch's activations (transposed), plus its head-group's weight slices, and
produces a partial pre-bias output out^T [D, L] (bf16).  Host sums the 4
partials per batch and adds bo_eff = bo + bv @ Wo (bv commutes through the
softmax-normalized attention, bk cancels in softmax entirely but is still
applied for exactness of the q/k biases).

v3 design vs the bf16 v2 baseline (190.9us):
  - q/k/v projections run as fp8 hi+lo DoubleRow matmuls: W is host-scaled
    by 32 (so the lo residuals stay in e4m3 normal range), split into
    Wh=fp8(32W), Wl=fp8(32W-Wh), and likewise X into Xh/Xl.  Each dt-pair
    contributes three DoubleRow matmuls (XhWh+XlWl fused diagonally is NOT
    used; terms are Wh*Xh, Wh*Xl, Wl*Xh across dt-pair subtiles), i.e.
    12 matmuls x 256 cyc per (chunk, group) instead of 8 x 512: -25% PE.
    Dropping the Wl*Xl term keeps bf16-level accuracy (ablated 6.7e-3 rel
    vs 8.1e-3 for all-bf16).  The x32 scale folds out exactly: exp scale
    *= 2^-10, host Wo *= 2^-5.
  - ScalarE runs ONLY the exp stream (128 activations of [128,1024]):
    normalization muls and PSUM->SBUF copies all move to DVE; tail copies
    (after the last exp) use ScalarE.  The exp stream is the 133us wall
    the schedule is built around.
  - Scores / attn@v / out-projection stay bf16 (fp8 ablates to >2e-2 rel:
    concentrated-attention rows do not average quantization noise away).
  - ident / onp / tps are bf16 (transpose cost 1.0 cyc/row vs 1.5 f32r).
  - Fill schedule rebalanced so phases B/C/D also carry projection /
    out-projection work and the PE never idles behind the exp stream.
"""

import numpy as np
import ml_dtypes

import concourse.mybir as mybir
import concourse.tile as tile
from concourse import bacc
from concourse.bass_utils import run_bass_kernel_spmd

F32 = mybir.dt.float32
BF16 = mybir.dt.bfloat16
F8 = mybir.dt.float8e4
DR = mybir.MatmulPerfMode.DoubleRow

WBOOST = 32.0  # host W scale; folds out via exp scale and Wo scale


def build_attention_core(L=2048, D=1024, H_LOC=4, HD=64, debug=False):
    JC = H_LOC * HD                   # local head channels (256)
    NJT = JC // 128                   # j-tiles (2)
    NDT = D // 128                    # contraction tiles (8)
    NPR = NDT // 2                    # dt-pairs for DoubleRow (4)
    NLT = L // 128                    # k-position tiles (16)
    XC = 512                          # x chunk width
    NXC = L // XC                     # x chunks per stream (4)
    QC = 1024                         # attention q chunk
    NQC = L // QC                     # 2
    NQT = QC // 128                   # q tiles per chunk (8)
    CS = 512                          # matmul N chunk
    # exp scale: 1/sqrt(HD) with the host-side W x32 boost (q,k each x32)
    SCALE = float(1.0 / (np.sqrt(HD) * WBOOST * WBOOST))

    nc = bacc.Bacc("TRN2", target_bir_lowering=False, debug=False, num_devices=8)

    xqh = nc.dram_tensor("xqh", [D, L], F8, kind="ExternalInput").ap()
    xql = nc.dram_tensor("xql", [D, L], F8, kind="ExternalInput").ap()
    xkh = nc.dram_tensor("xkh", [D, L], F8, kind="ExternalInput").ap()
    xkl = nc.dram_tensor("xkl", [D, L], F8, kind="ExternalInput").ap()
    xvh = nc.dram_tensor("xvh", [D, L], F8, kind="ExternalInput").ap()
    xvl = nc.dram_tensor("xvl", [D, L], F8, kind="ExternalInput").ap()
    wqh = nc.dram_tensor("wqh", [D, JC], F8, kind="ExternalInput").ap()
    wql = nc.dram_tensor("wql", [D, JC], F8, kind="ExternalInput").ap()
    wkh = nc.dram_tensor("wkh", [D, JC], F8, kind="ExternalInput").ap()
    wkl = nc.dram_tensor("wkl", [D, JC], F8, kind="ExternalInput").ap()
    wvh = nc.dram_tensor("wvh", [D, JC], F8, kind="ExternalInput").ap()
    wvl = nc.dram_tensor("wvl", [D, JC], F8, kind="ExternalInput").ap()
    wo = nc.dram_tensor("wo", [JC, D], BF16, kind="ExternalInput").ap()
    bq = nc.dram_tensor("bq", [1, JC], F32, kind="ExternalInput").ap()
    bk = nc.dram_tensor("bk", [1, JC], F32, kind="ExternalInput").ap()
    ident = nc.dram_tensor("ident", [128, 128], BF16, kind="ExternalInput").ap()
    outT = nc.dram_tensor("outT", [D, L], BF16, kind="ExternalOutput").ap()

    from contextlib import ExitStack

    with (
        tile.TileContext(nc) as tc,
        ExitStack() as stack,
        nc.allow_low_precision(reason="bf16/fp8-hilo operands; psum accum is f32"),
    ):
        consts = stack.enter_context(tc.tile_pool(name="consts", bufs=1))
        wpool = stack.enter_context(tc.tile_pool(name="wpool", bufs=1))
        prod = stack.enter_context(tc.tile_pool(name="prod", bufs=1))
        xt_pool = stack.enter_context(tc.tile_pool(name="xt", bufs=24))
        att_pool = stack.enter_context(tc.tile_pool(name="att", bufs=6))
        onp_pool = stack.enter_context(tc.tile_pool(name="onp", bufs=2))
        rd_pool = stack.enter_context(tc.tile_pool(name="rd", bufs=2))
        oc_pool = stack.enter_context(tc.tile_pool(name="oc", bufs=8))
        # PSUM banks: st0/st1 [128,1024] (2 each) + acc0/acc1/acc2 (1 each,
        # 65-wide attn@v regions packed 7 per bank) + sp [128,512] (1) = 8.
        psum = stack.enter_context(tc.tile_pool(name="psum", bufs=1, space="PSUM"))

        warm_row = consts.tile([1, CS], BF16)    # PE p-state warmup stream
        nc.vector.memset(warm_row, 0.0)
        warm_one = consts.tile([1, 128], BF16)
        nc.vector.memset(warm_one, 0.0)
        vones = consts.tile([128, NLT * H_LOC], BF16)
        nc.vector.memset(vones, 1.0)

        wq_sb = [wpool.tile([128, NDT, JC], F8, tag=f"wq{i}", name="wq")
                 for i in range(2)]
        wk_sb = [wpool.tile([128, NDT, JC], F8, tag=f"wk{i}", name="wk")
                 for i in range(2)]
        wv_sb = [wpool.tile([128, NDT, JC], F8, tag=f"wv{i}", name="wv")
                 for i in range(2)]
        wo_sb = wpool.tile([128, NJT, D], BF16, tag="wo")
        bq_col = wpool.tile([128, NJT], F32, tag="bq")
        bk_col = wpool.tile([128, NJT], F32, tag="bk")
        ident_sb = wpool.tile([128, 128], BF16, tag="ident")

        kT_sb = prod.tile([128, NJT, L], BF16, tag="kT")   # [j%128, jt, kpos]
        qT_sb = prod.tile([128, NJT, L], BF16, tag="qT")
        v_sb = prod.tile([128, NLT, H_LOC, HD + 1], BF16, tag="v")  # [kpos%128, kt, h, ch|1]
        onT_sb = prod.tile([128, NJT, L], BF16, tag="onT")  # [j%128, jt, qpos]
        nc.vector.tensor_copy(
            out=v_sb[:, :, :, HD:HD + 1].rearrange("p a h c -> p (a h c)"),
            in_=vones,
        )

        # ---- emission helpers -------------------------------------------
        def load_x(eng, x_h, x_l, c):
            """Two-DMA chunk load: x^T hi/lo [:, c*XC:(c+1)*XC] -> [128, NDT, XC]."""
            ts = []
            for x_dram in (x_h, x_l):
                x_t = xt_pool.tile([128, NDT, XC], F8, tag="xt", name="xt")
                eng.dma_start(
                    out=x_t,
                    in_=x_dram.rearrange("(t p) l -> p t l", p=128)[
                        :, :, c * XC:(c + 1) * XC],
                )
                ts.append(x_t)
            return ts

        def kqproj_g(w_hl, b_col, dst, x_hl, c, g, tag=None):
            """dst[j, l] = sum_d W[d, j] x^T[d, l] + b[j] for one (chunk, group).
            fp8 hi-lo DoubleRow over dt-pairs: Wh*Xh + Wh*Xl + Wl*Xh."""
            tag = tag or "sp"
            shape = [128, QC] if tag.startswith("st") else [128, CS]
            ps = psum.tile(shape, F32, tag=tag, name="pj")
            w_h, w_l = w_hl
            x_h, x_l = x_hl
            js = slice(g * 128, (g + 1) * 128)
            for t in range(NPR):
                sl = slice(2 * t, 2 * t + 2)
                nc.tensor.matmul(ps[:, 0:XC], w_h[:, sl, js], x_h[:, sl, :],
                                 start=(t == 0), stop=False, perf_mode=DR)
                nc.tensor.matmul(ps[:, 0:XC], w_h[:, sl, js], x_l[:, sl, :],
                                 start=False, stop=False, perf_mode=DR)
                nc.tensor.matmul(ps[:, 0:XC], w_l[:, sl, js], x_h[:, sl, :],
                                 start=False, stop=(t == NPR - 1), perf_mode=DR)
            nc.vector.tensor_scalar_add(
                dst[:, g, c * XC:(c + 1) * XC], ps[:, 0:XC], b_col[:, g:g + 1],
            )

        def vproj_lt(x_hl, c, lt):
            """v[kpos, j] = sum_d x^T[d, kpos] W[d, j], one 128-row tile.
            fp8 hi-lo DoubleRow; bv is folded into bo on the host."""
            ps = psum.tile([128, CS], F32, tag="sp", name="pv")
            x_h, x_l = x_hl
            off = lt * 128 - c * XC
            ks = slice(off, off + 128)
            for t in range(NPR):
                sl = slice(2 * t, 2 * t + 2)
                nc.tensor.matmul(ps[:, 0:JC], x_h[:, sl, ks], wv_sb[0][:, sl, :],
                                 start=(t == 0), stop=False, perf_mode=DR)
                nc.tensor.matmul(ps[:, 0:JC], x_h[:, sl, ks], wv_sb[1][:, sl, :],
                                 start=False, stop=False, perf_mode=DR)
                nc.tensor.matmul(ps[:, 0:JC], x_l[:, sl, ks], wv_sb[0][:, sl, :],
                                 start=False, stop=(t == NPR - 1), perf_mode=DR)
            nc.vector.tensor_copy(
                out=v_sb[:, lt, :, 0:HD],
                in_=ps[:, 0:JC].rearrange("p (h c) -> p h c", h=H_LOC),
            )

        def score(h, qc, kt):
            jt, hp = h // 2, (h % 2) * HD
            st = psum.tile([128, QC], F32, tag=f"st{h % 2}", name=f"st{h % 2}")
            for cs in range(QC // CS):
                nc.tensor.matmul(
                    st[:, cs * CS:(cs + 1) * CS],
                    kT_sb[hp:hp + HD, jt, kt * 128:kt * 128 + 128],
                    qT_sb[hp:hp + HD, jt, qc * QC + cs * CS:qc * QC + (cs + 1) * CS],
                    start=True,
                    stop=True,
                )
            return st

        def exp_(h, st):
            at = att_pool.tile([128, QC], BF16, tag=f"at{h % 2}", name=f"at{h % 2}")
            nc.scalar.activation(
                out=at, in_=st, func=mybir.ActivationFunctionType.Exp, scale=SCALE,
            )
            return at

        # acc region map: r = h2*8 + qt -> psum tile r//7, 65-col slot r%7.
        # (65-wide regions packed 7 per 2KB bank so none straddles a bank.)
        ACC_FIRST = (0, 7, 14)
        ACC_LAST = (6, 13, 15)

        def attnv(h, kt, at, accs):
            # HW semantics: a start=True matmul zeroes the ENTIRE psum bank,
            # so exactly one start per bank (first write) and one stop (last).
            for qt in range(NQT):
                r = (h % 2) * NQT + qt
                t, slot = divmod(r, 7)
                nc.tensor.matmul(
                    accs[t][:, slot * 65:slot * 65 + 65],
                    at[:, qt * 128:(qt + 1) * 128],
                    v_sb[:, kt, h, :],
                    start=(kt == 0 and r == ACC_FIRST[t]),
                    stop=(kt == NLT - 1 and r == ACC_LAST[t]),
                    skip_group_check=True,
                )

        def epilogue(pair, qc, accs, mid=None):
            """onorm_pair[q, qt, h2*64+ch] = acc * recip(den-col); PE-transpose
            each [128,128] q x (h2,ch) tile into onT[j, qpos]."""
            rdens = []
            for t, n in ((0, 7), (1, 7), (2, 2)):
                rd = rd_pool.tile([128, n], F32, tag=f"rd{t}", name="rd")
                nc.vector.reciprocal(out=rd, in_=accs[t][:, 64::65][:, 0:n])
                rdens.append(rd)
            onp = onp_pool.tile([128, NQT, 128], BF16, tag="onp", name="onp")
            tps = [psum.tile([128, NQT // 2, 128], BF16, tag=t, name="tp")
                   for t in ("sp", "acc2")]
            for qt in range(NQT):
                for h2 in range(2):
                    r = h2 * NQT + qt
                    t, slot = divmod(r, 7)
                    nc.vector.tensor_scalar_mul(
                        onp[:, qt, h2 * HD:(h2 + 1) * HD],
                        accs[t][:, slot * 65:slot * 65 + 64],
                        rdens[t][:, slot:slot + 1],
                    )
                half, q4 = divmod(qt, NQT // 2)
                nc.tensor.matmul(
                    tps[half][:, q4, :], onp[:, qt, :], ident_sb,
                    is_transpose=True, start=(q4 == 0), stop=(q4 == NQT // 2 - 1),
                    skip_group_check=True,
                )
                if q4 == NQT // 2 - 1:
                    nc.vector.tensor_copy(
                        out=onT_sb[:, pair,
                                   qc * QC + half * (QC // 2):
                                   qc * QC + (half + 1) * (QC // 2)],
                        in_=tps[half].rearrange("p a b -> p (a b)"),
                    )
                    if mid is not None and half == 0:
                        mid()

        def make_accs():
            return [psum.tile([128, w], F32, tag=f"acc{i}", name=f"acc{i}")
                    for i, w in ((0, 455), (1, 455), (2, 130))]

        def outproj_chunk(qc, mt, cs2, tag="sp", copy_eng="vector",
                          dma_eng=None):
            shape = [128, QC] if tag.startswith("st") else [128, CS]
            ps = psum.tile(shape, F32, tag=tag, name="po")
            for jt in range(NJT):
                nc.tensor.matmul(
                    ps[:, 0:CS],
                    wo_sb[:, jt, mt * 128:mt * 128 + 128],
                    onT_sb[:, jt, qc * QC + cs2 * CS:qc * QC + (cs2 + 1) * CS],
                    start=(jt == 0),
                    stop=(jt == NJT - 1),
                )
            ob = oc_pool.tile([128, CS], BF16, tag="oc", name="oc")
            if copy_eng == "scalar":
                nc.scalar.copy(out=ob, in_=ps[:, 0:CS])
            else:
                nc.vector.tensor_copy(out=ob, in_=ps[:, 0:CS])
            (dma_eng or nc.sync).dma_start(
                out=outT[mt * 128:mt * 128 + 128,
                         qc * QC + cs2 * CS:qc * QC + (cs2 + 1) * CS],
                in_=ob,
            )

        # ---- emission schedule ------------------------------------------
        # DMA queues: SP (hwdge) carries q/k streams + q/k weights in
        # deadline order; the otherwise idle GpSimd swdge queue carries the
        # v stream + wv + wo.  Out-DMAs ride SP/Activation hwdge.
        nc.sync.dma_start(out=wq_sb[0], in_=wqh.rearrange("(t p) j -> p t j", p=128))
        nc.sync.dma_start(out=wq_sb[1], in_=wql.rearrange("(t p) j -> p t j", p=128))
        nc.sync.dma_start(out=bq_col, in_=bq.rearrange("a (t p) -> p (a t)", p=128))

        xk = [None] * NXC
        xq = [None] * NXC
        xv = [None] * NXC
        nc.gpsimd.dma_start(out=wv_sb[0], in_=wvh.rearrange("(t p) j -> p t j", p=128))
        nc.gpsimd.dma_start(out=wv_sb[1], in_=wvl.rearrange("(t p) j -> p t j", p=128))
        xv[0] = load_x(nc.gpsimd, xvh, xvl, 0)
        xq[0] = load_x(nc.sync, xqh, xql, 0)
        xq[1] = load_x(nc.sync, xqh, xql, 1)
        nc.sync.dma_start(out=wk_sb[0], in_=wkh.rearrange("(t p) j -> p t j", p=128))
        nc.sync.dma_start(out=wk_sb[1], in_=wkl.rearrange("(t p) j -> p t j", p=128))
        nc.sync.dma_start(out=bk_col, in_=bk.rearrange("a (t p) -> p (a t)", p=128))
        nc.sync.dma_start(out=ident_sb, in_=ident)
        xk[0] = load_x(nc.sync, xkh, xkl, 0)
        xv[1] = load_x(nc.gpsimd, xvh, xvl, 1)
        xk[1] = load_x(nc.sync, xkh, xkl, 1)
        xq[2] = load_x(nc.sync, xqh, xql, 2)
        xv[2] = load_x(nc.gpsimd, xvh, xvl, 2)
        xk[2] = load_x(nc.sync, xkh, xkl, 2)
        xq[3] = load_x(nc.sync, xqh, xql, 3)
        xv[3] = load_x(nc.gpsimd, xvh, xvl, 3)
        xk[3] = load_x(nc.sync, xkh, xkl, 3)
        nc.gpsimd.dma_start(out=wo_sb, in_=wo.rearrange("(t p) d -> p t d", p=128))

        # PE p-state warmup: a stream of throwaway matmuls keeps the PE busy
        # while the first x chunks stream in, so the real projections start
        # at full clock instead of pstate-low.
        warm_ps = psum.tile([128, CS], F32, tag="sp", name="warm")
        for _ in range(11):
            nc.tensor.matmul(warm_ps, warm_one, warm_row, start=True, stop=True,
                             skip_group_check=True)

        # pre-attention projections: phase A (qc0, pair0) only needs the
        # group-0 rows of qT (chunks 0,1) and kT (chunk 0); the group-1 and
        # later-chunk projections ride the fill slots.
        kqproj_g((wq_sb[0], wq_sb[1]), bq_col, qT_sb, xq[0], 0, 0, tag="acc0")
        kqproj_g((wq_sb[0], wq_sb[1]), bq_col, qT_sb, xq[1], 1, 0, tag="acc1")
        kqproj_g((wk_sb[0], wk_sb[1]), bk_col, kT_sb, xk[0], 0, 0, tag="acc2")

        # fill schedule: fills[phase][kt] = list of work items emitted after
        # that kt's scores/exps/attn@v.  Phase order is A=(qc0,p0),
        # B=(qc1,p0), C=(qc0,p1), D=(qc1,p1) so that B reuses group-0
        # kT/qT rows and group-1 projections spread into B/C.  Deadlines:
        # vproj(lt) in a slot <= lt; kproj (c,0) by A-slot 4c-1; qproj
        # (c2/c3, 0) by end of A (B reads qc1); kproj (*,1) + qproj
        # (c0/c1, 1) by end of B; qproj (c2/c3, 1) by end of C;
        # out-projection of qc0 fills D (needs C's epilogue).
        K, Q, V, O = "kproj", "qproj", "vproj", "outproj"
        slotsA = [[] for _ in range(NLT)]
        for lt in range(NLT):
            slotsA[lt].append((V, (lt // 4, lt)))
        for c in (1, 2, 3):
            slotsA[4 * (c - 1) + 2].append((K, (c, 0)))
        slotsA[5].append((Q, (2, 0)))
        slotsA[9].append((Q, (3, 0)))
        slotsB = [[] for _ in range(NLT)]
        slotsB[0].append((K, (0, 1)))
        slotsB[3].append((K, (1, 1)))
        slotsB[6].append((K, (2, 1)))
        slotsB[9].append((K, (3, 1)))
        slotsB[11].append((Q, (0, 1)))
        slotsB[13].append((Q, (1, 1)))
        slotsC = [[] for _ in range(NLT)]
        slotsC[1].append((Q, (2, 1)))
        slotsC[5].append((Q, (3, 1)))
        # outproj qc0 needs C's epilogue (lands ~kt0 of D): start at slot 2
        # and double up the last two slots.
        slotsD = [[] for _ in range(NLT)]
        d_order = list(range(2, 16)) + [14, 15]
        for i in range(16):
            slotsD[d_order[i]].append((O, (0, i // 2, i % 2)))
        fill_slots = [slotsA, slotsB, slotsC, slotsD]

        def run_fill(kind, args):
            if kind == K:
                c, g = args
                kqproj_g((wk_sb[0], wk_sb[1]), bk_col, kT_sb, xk[c], c, g, tag="sp")
            elif kind == Q:
                c, g = args
                kqproj_g((wq_sb[0], wq_sb[1]), bq_col, qT_sb, xq[c], c, g, tag="sp")
            elif kind == V:
                c, lt = args
                vproj_lt(xv[c], c, lt)
            else:
                outproj_chunk(*args)

        pending_epi = None
        for phase, (qc, pair) in enumerate([(0, 0), (1, 0), (0, 1), (1, 1)]):
            slots = fill_slots[phase]
            h0, h1 = 2 * pair, 2 * pair + 1
            accs = None
            prev = None
            for kt in range(NLT):
                st0 = score(h0, qc, kt)
                at0 = exp_(h0, st0)
                if pending_epi is not None:
                    # previous phase's epilogue rides behind this phase's
                    # first scores so they aren't queued after its transposes
                    epilogue(*pending_epi)
                    pending_epi = None
                if accs is None:
                    accs = make_accs()
                if prev is not None:
                    attnv(h0, kt - 1, prev[0], accs)
                st1 = score(h1, qc, kt)
                at1 = exp_(h1, st1)
                if prev is not None:
                    attnv(h1, kt - 1, prev[1], accs)
                prev = (at0, at1)
                for item in slots[kt]:
                    run_fill(*item)
            attnv(h0, NLT - 1, prev[0], accs)
            attnv(h1, NLT - 1, prev[1], accs)
            pending_epi = (pair, qc, accs)

        # tail: out-projection of qc1 - the first half rides inside the last
        # epilogue (right after the qt0-3 transposes land in onT)
        tags = ["st0", "st1", "acc0", "acc1"]

        def _tail_first_half():
            for i, mt in enumerate(range(NDT)):
                outproj_chunk(1, mt, 0, tag=tags[i % len(tags)],
                              copy_eng="scalar" if i % 2 else "vector",
                              dma_eng=nc.scalar if i % 2 else nc.sync)

        epilogue(*pending_epi, mid=_tail_first_half)
        for i, mt in enumerate(range(NDT)):
            outproj_chunk(1, mt, 1, tag=tags[i % len(tags)],
                          copy_eng="scalar" if i % 2 else "vector",
                          dma_eng=nc.scalar if i % 2 else nc.sync)

    nc.compile()
    return nc


_NC_CACHE = {}


def _get_nc():
    if "nc" not in _NC_CACHE:
        _NC_CACHE["nc"] = build_attention_core()
    return _NC_CACHE["nc"]


def _bf16(x):
    return np.asarray(x, np.float32).astype(ml_dtypes.bfloat16)


def _hilo(x):
    """fp8e4m3 hi+lo split (IEEE e4m3: bias 8, max 240)."""
    xh = np.asarray(x, np.float32).astype(ml_dtypes.float8_e4m3)
    xl = (np.asarray(x, np.float32) - xh.astype(np.float32)).astype(
        ml_dtypes.float8_e4m3)
    return np.ascontiguousarray(xh), np.ascontiguousarray(xl)


def shard_inputs(query, key_, value, Wq, bq, Wk, bk, Wv, bv, Wo, bo,
                 B=2, H=16, H_LOC=4, HD=64):
    """Host-side sharding: core c -> (batch c//4, head-group c%4)."""
    groups = H // H_LOC
    xT = [np.ascontiguousarray(np.asarray(x, np.float32).transpose(0, 2, 1))
          for x in (query, key_, value)]
    xT_hl = [[_hilo(xT[s][b]) for b in range(B)] for s in range(3)]
    Wq, Wk, Wv, Wo = (np.asarray(w, np.float32) for w in (Wq, Wk, Wv, Wo))
    bq, bk = (np.asarray(b, np.float32) for b in (bq, bk))
    ident = np.eye(128, dtype=np.float32).astype(ml_dtypes.bfloat16)
    in_maps = []
    for c in range(B * groups):
        b, g = divmod(c, groups)
        js = slice(g * H_LOC * HD, (g + 1) * H_LOC * HD)
        qh, ql = _hilo(Wq[:, js] * WBOOST)
        kh, kl = _hilo(Wk[:, js] * WBOOST)
        vh, vl = _hilo(Wv[:, js] * WBOOST)
        in_maps.append({
            "xqh": xT_hl[0][b][0], "xql": xT_hl[0][b][1],
            "xkh": xT_hl[1][b][0], "xkl": xT_hl[1][b][1],
            "xvh": xT_hl[2][b][0], "xvl": xT_hl[2][b][1],
            "wqh": qh, "wql": ql,
            "wkh": kh, "wkl": kl,
            "wvh": vh, "wvl": vl,
            "wo": _bf16(np.ascontiguousarray(Wo[js, :]) / WBOOST),
            "bq": np.ascontiguousarray(bq[None, js]) * WBOOST,
            "bk": np.ascontiguousarray(bk[None, js]) * WBOOST,
            "ident": ident,
        })
    return in_maps


def kernel(query, key_, value, Wq, bq, Wk, bk, Wv, bv, Wo, bo):
    B, L, D = 2, 2048, 1024
    groups = 4
    nc = _get_nc()
    in_maps = shard_inputs(query, key_, value, Wq, bq, Wk, bk, Wv, bv, Wo, bo)
    res = run_bass_kernel_spmd(nc, in_maps, list(range(8))).results
    out = np.empty((B, L, D), np.float32)
    # bv commutes through the normalized attention: out += (bv @ Wo + bo)
    bo_eff = (np.asarray(bv, np.float32) @ np.asarray(Wo, np.float32)
              + np.asarray(bo, np.float32))
    for b in range(B):
        acc = res[b * groups]["outT"].astype(np.float32)
        for g in range(1, groups):
            acc = acc + res[b * groups + g]["outT"]
        out[b] = acc.T + bo_eff
    return out


# revision 8
# speedup vs baseline: 1.0772x; 1.0772x over previous
"""Memory-efficient multi-head attention on 8 TRN2 NeuronCores.

Problem (hardcoded): B=2, L=2048, D=1024, H=16, HD=64.
  out = softmax((X_q Wq + bq)(X_k Wk + bk)^T / sqrt(HD)) (X_v Wv + bv) Wo + bo

Sharding: 8 cores = 2 batches x 4 head-groups (4 heads each).  Each core gets
its batch's activations (transposed), plus its head-group's weight slices, and
produces a partial pre-bias output out^T [D, L] (bf16).  Host sums the 4
partials per batch and adds bo_eff = bo + bv @ Wo (bv commutes through the
softmax-normalized attention, bk cancels in softmax entirely but is still
applied for exactness of the q/k biases).

v3 design vs the bf16 v2 baseline (190.9us):
  - q/k/v projections run as fp8 hi+lo DoubleRow matmuls: W is host-scaled
    by 32 (so the lo residuals stay in e4m3 normal range), split into
    Wh=fp8(32W), Wl=fp8(32W-Wh), and likewise X into Xh/Xl.  Each dt-pair
    contributes three DoubleRow matmuls (XhWh+XlWl fused diagonally is NOT
    used; terms are Wh*Xh, Wh*Xl, Wl*Xh across dt-pair subtiles), i.e.
    12 matmuls x 256 cyc per (chunk, group) instead of 8 x 512: -25% PE.
    Dropping the Wl*Xl term keeps bf16-level accuracy (ablated 6.7e-3 rel
    vs 8.1e-3 for all-bf16).  The x32 scale folds out exactly: exp scale
    *= 2^-10, host Wo *= 2^-5.
  - ScalarE runs ONLY the exp stream (128 activations of [128,1024]):
    normalization muls and PSUM->SBUF copies all move to DVE; tail copies
    (after the last exp) use ScalarE.  The exp stream is the 133us wall
    the schedule is built around.
  - Scores / attn@v / out-projection stay bf16 (fp8 ablates to >2e-2 rel:
    concentrated-attention rows do not average quantization noise away).
  - ident / onp / tps are bf16 (transpose cost 1.0 cyc/row vs 1.5 f32r).
  - Fill schedule rebalanced so phases B/C/D also carry projection /
    out-projection work and the PE never idles behind the exp stream.
"""

import numpy as np
import ml_dtypes

import concourse.mybir as mybir
import concourse.tile as tile
from concourse import bacc
from concourse.bass_utils import run_bass_kernel_spmd

F32 = mybir.dt.float32
BF16 = mybir.dt.bfloat16
F8 = mybir.dt.float8e4
DR = mybir.MatmulPerfMode.DoubleRow

WBOOST = 32.0  # host W scale; folds out via exp scale and Wo scale


def build_attention_core(L=2048, D=1024, H_LOC=4, HD=64, debug=False):
    JC = H_LOC * HD                   # local head channels (256)
    NJT = JC // 128                   # j-tiles (2)
    NDT = D // 128                    # contraction tiles (8)
    NPR = NDT // 2                    # dt-pairs for DoubleRow (4)
    NLT = L // 128                    # k-position tiles (16)
    XC = 512                          # x chunk width
    NXC = L // XC                     # x chunks per stream (4)
    QC = 1024                         # attention q chunk
    NQC = L // QC                     # 2
    NQT = QC // 128                   # q tiles per chunk (8)
    CS = 512                          # matmul N chunk
    # exp scale: 1/sqrt(HD) with the host-side W x32 boost (q,k each x32)
    SCALE = float(1.0 / (np.sqrt(HD) * WBOOST * WBOOST))

    nc = bacc.Bacc("TRN2", target_bir_lowering=False, debug=False, num_devices=8)

    xqh = nc.dram_tensor("xqh", [D, L], F8, kind="ExternalInput").ap()
    xql = nc.dram_tensor("xql", [D, L], F8, kind="ExternalInput").ap()
    xkh = nc.dram_tensor("xkh", [D, L], F8, kind="ExternalInput").ap()
    xkl = nc.dram_tensor("xkl", [D, L], F8, kind="ExternalInput").ap()
    xvh = nc.dram_tensor("xvh", [D, L], F8, kind="ExternalInput").ap()
    xvl = nc.dram_tensor("xvl", [D, L], F8, kind="ExternalInput").ap()
    wqh = nc.dram_tensor("wqh", [D, JC], F8, kind="ExternalInput").ap()
    wql = nc.dram_tensor("wql", [D, JC], F8, kind="ExternalInput").ap()
    wkh = nc.dram_tensor("wkh", [D, JC], F8, kind="ExternalInput").ap()
    wkl = nc.dram_tensor("wkl", [D, JC], F8, kind="ExternalInput").ap()
    wvh = nc.dram_tensor("wvh", [D, JC], F8, kind="ExternalInput").ap()
    wvl = nc.dram_tensor("wvl", [D, JC], F8, kind="ExternalInput").ap()
    wo = nc.dram_tensor("wo", [JC, D], BF16, kind="ExternalInput").ap()
    bq = nc.dram_tensor("bq", [1, JC], F32, kind="ExternalInput").ap()
    bk = nc.dram_tensor("bk", [1, JC], F32, kind="ExternalInput").ap()
    ident = nc.dram_tensor("ident", [128, 128], BF16, kind="ExternalInput").ap()
    outT = nc.dram_tensor("outT", [D, L], BF16, kind="ExternalOutput").ap()

    from contextlib import ExitStack

    with (
        tile.TileContext(nc) as tc,
        ExitStack() as stack,
        nc.allow_low_precision(reason="bf16/fp8-hilo operands; psum accum is f32"),
    ):
        consts = stack.enter_context(tc.tile_pool(name="consts", bufs=1))
        wpool = stack.enter_context(tc.tile_pool(name="wpool", bufs=1))
        prod = stack.enter_context(tc.tile_pool(name="prod", bufs=1))
        xt_pool = stack.enter_context(tc.tile_pool(name="xt", bufs=24))
        att_pool = stack.enter_context(tc.tile_pool(name="att", bufs=6))
        onp_pool = stack.enter_context(tc.tile_pool(name="onp", bufs=2))
        rd_pool = stack.enter_context(tc.tile_pool(name="rd", bufs=2))
        oc_pool = stack.enter_context(tc.tile_pool(name="oc", bufs=8))
        # PSUM banks: st0/st1 [128,1024] (2 each) + acc0/acc1/acc2 (1 each,
        # 65-wide attn@v regions packed 7 per bank) + sp [128,512] (1) = 8.
        psum = stack.enter_context(tc.tile_pool(name="psum", bufs=1, space="PSUM"))

        warm_row = consts.tile([1, CS], BF16)    # PE p-state warmup stream
        nc.vector.memset(warm_row, 0.0)
        warm_one = consts.tile([1, 128], BF16)
        nc.vector.memset(warm_one, 0.0)
        vones = consts.tile([128, NLT * H_LOC], BF16)
        nc.vector.memset(vones, 1.0)

        wq_sb = [wpool.tile([128, NDT, JC], F8, tag=f"wq{i}", name="wq")
                 for i in range(2)]
        wk_sb = [wpool.tile([128, NDT, JC], F8, tag=f"wk{i}", name="wk")
                 for i in range(2)]
        wv_sb = [wpool.tile([128, NDT, JC], F8, tag=f"wv{i}", name="wv")
                 for i in range(2)]
        wo_sb = wpool.tile([128, NJT, D], BF16, tag="wo")
        bq_col = wpool.tile([128, NJT], F32, tag="bq")
        bk_col = wpool.tile([128, NJT], F32, tag="bk")
        ident_sb = wpool.tile([128, 128], BF16, tag="ident")

        kT_sb = prod.tile([128, NJT, L], BF16, tag="kT")   # [j%128, jt, kpos]
        qT_sb = prod.tile([128, NJT, L], BF16, tag="qT")
        v_sb = prod.tile([128, NLT, H_LOC, HD + 1], BF16, tag="v")  # [kpos%128, kt, h, ch|1]
        onT_sb = prod.tile([128, NJT, L], BF16, tag="onT")  # [j%128, jt, qpos]
        nc.vector.tensor_copy(
            out=v_sb[:, :, :, HD:HD + 1].rearrange("p a h c -> p (a h c)"),
            in_=vones,
        )

        # ---- emission helpers -------------------------------------------
        def load_x(eng, x_h, x_l, c):
            """Two-DMA chunk load: x^T hi/lo [:, c*XC:(c+1)*XC] -> [128, NDT, XC]."""
            ts = []
            for x_dram in (x_h, x_l):
                x_t = xt_pool.tile([128, NDT, XC], F8, tag="xt", name="xt")
                eng.dma_start(
                    out=x_t,
                    in_=x_dram.rearrange("(t p) l -> p t l", p=128)[
                        :, :, c * XC:(c + 1) * XC],
                )
                ts.append(x_t)
            return ts

        def kqproj_g(w_hl, b_col, dst, x_hl, c, g, tag=None):
            """dst[j, l] = sum_d W[d, j] x^T[d, l] + b[j] for one (chunk, group).
            fp8 hi-lo DoubleRow over dt-pairs: Wh*Xh + Wh*Xl + Wl*Xh."""
            tag = tag or "sp"
            shape = [128, QC] if tag.startswith("st") else [128, CS]
            ps = psum.tile(shape, F32, tag=tag, name="pj")
            w_h, w_l = w_hl
            x_h, x_l = x_hl
            js = slice(g * 128, (g + 1) * 128)
            for t in range(NPR):
                sl = slice(2 * t, 2 * t + 2)
                nc.tensor.matmul(ps[:, 0:XC], w_h[:, sl, js], x_h[:, sl, :],
                                 start=(t == 0), stop=False, perf_mode=DR)
                nc.tensor.matmul(ps[:, 0:XC], w_h[:, sl, js], x_l[:, sl, :],
                                 start=False, stop=False, perf_mode=DR)
                nc.tensor.matmul(ps[:, 0:XC], w_l[:, sl, js], x_h[:, sl, :],
                                 start=False, stop=(t == NPR - 1), perf_mode=DR)
            nc.vector.tensor_scalar_add(
                dst[:, g, c * XC:(c + 1) * XC], ps[:, 0:XC], b_col[:, g:g + 1],
            )

        def vproj_lt(x_hl, c, lt):
            """v[kpos, j] = sum_d x^T[d, kpos] W[d, j], one 128-row tile.
            fp8 hi-lo DoubleRow; bv is folded into bo on the host."""
            ps = psum.tile([128, CS], F32, tag="sp", name="pv")
            x_h, x_l = x_hl
            off = lt * 128 - c * XC
            ks = slice(off, off + 128)
            for t in range(NPR):
                sl = slice(2 * t, 2 * t + 2)
                nc.tensor.matmul(ps[:, 0:JC], x_h[:, sl, ks], wv_sb[0][:, sl, :],
                                 start=(t == 0), stop=False, perf_mode=DR)
                nc.tensor.matmul(ps[:, 0:JC], x_h[:, sl, ks], wv_sb[1][:, sl, :],
                                 start=False, stop=False, perf_mode=DR)
                nc.tensor.matmul(ps[:, 0:JC], x_l[:, sl, ks], wv_sb[0][:, sl, :],
                                 start=False, stop=(t == NPR - 1), perf_mode=DR)
            nc.vector.tensor_copy(
                out=v_sb[:, lt, :, 0:HD],
                in_=ps[:, 0:JC].rearrange("p (h c) -> p h c", h=H_LOC),
            )

        def score(h, qc, kt):
            jt, hp = h // 2, (h % 2) * HD
            st = psum.tile([128, QC], F32, tag=f"st{h % 2}", name=f"st{h % 2}")
            for cs in range(QC // CS):
                nc.tensor.matmul(
                    st[:, cs * CS:(cs + 1) * CS],
                    kT_sb[hp:hp + HD, jt, kt * 128:kt * 128 + 128],
                    qT_sb[hp:hp + HD, jt, qc * QC + cs * CS:qc * QC + (cs + 1) * CS],
                    start=True,
                    stop=True,
                )
            return st

        def exp_(h, st):
            at = att_pool.tile([128, QC], BF16, tag=f"at{h % 2}", name=f"at{h % 2}")
            nc.scalar.activation(
                out=at, in_=st, func=mybir.ActivationFunctionType.Exp, scale=SCALE,
            )
            return at

        # acc region map: r = h2*8 + qt -> psum tile r//7, 65-col slot r%7.
        # (65-wide regions packed 7 per 2KB bank so none straddles a bank.)
        ACC_FIRST = (0, 7, 14)
        ACC_LAST = (6, 13, 15)

        def attnv(h, kt, at, accs):
            # HW semantics: a start=True matmul zeroes the ENTIRE psum bank,
            # so exactly one start per bank (first write) and one stop (last).
            for qt in range(NQT):
                r = (h % 2) * NQT + qt
                t, slot = divmod(r, 7)
                nc.tensor.matmul(
                    accs[t][:, slot * 65:slot * 65 + 65],
                    at[:, qt * 128:(qt + 1) * 128],
                    v_sb[:, kt, h, :],
                    start=(kt == 0 and r == ACC_FIRST[t]),
                    stop=(kt == NLT - 1 and r == ACC_LAST[t]),
                    skip_group_check=True,
                )

        def epilogue(pair, qc, accs, mid=None):
            """onorm_pair[q, qt, h2*64+ch] = acc * recip(den-col); PE-transpose
            each [128,128] q x (h2,ch) tile into onT[j, qpos]."""
            rdens = []
            for t, n in ((0, 7), (1, 7), (2, 2)):
                rd = rd_pool.tile([128, n], F32, tag=f"rd{t}", name="rd")
                nc.vector.reciprocal(out=rd, in_=accs[t][:, 64::65][:, 0:n])
                rdens.append(rd)
            onp = onp_pool.tile([128, NQT, 128], BF16, tag="onp", name="onp")
            tps = [psum.tile([128, NQT // 2, 128], BF16, tag=t, name="tp")
                   for t in ("sp", "acc2")]
            for qt in range(NQT):
                for h2 in range(2):
                    r = h2 * NQT + qt
                    t, slot = divmod(r, 7)
                    nc.vector.tensor_scalar_mul(
                        onp[:, qt, h2 * HD:(h2 + 1) * HD],
                        accs[t][:, slot * 65:slot * 65 + 64],
                        rdens[t][:, slot:slot + 1],
                    )
                half, q4 = divmod(qt, NQT // 2)
                nc.tensor.matmul(
                    tps[half][:, q4, :], onp[:, qt, :], ident_sb,
                    is_transpose=True, start=(q4 == 0), stop=(q4 == NQT // 2 - 1),
                    skip_group_check=True,
                )
                if q4 == NQT // 2 - 1:
                    nc.vector.tensor_copy(
                        out=onT_sb[:, pair,
                                   qc * QC + half * (QC // 2):
                                   qc * QC + (half + 1) * (QC // 2)],
                        in_=tps[half].rearrange("p a b -> p (a b)"),
                    )
                    if mid is not None and half == 0:
                        mid()

        def make_accs():
            return [psum.tile([128, w], F32, tag=f"acc{i}", name=f"acc{i}")
                    for i, w in ((0, 455), (1, 455), (2, 130))]

        def outproj_chunk(qc, mt, cs2, tag="sp", copy_eng="vector",
                          dma_eng=None):
            shape = [128, QC] if tag.startswith("st") else [128, CS]
            ps = psum.tile(shape, F32, tag=tag, name="po")
            for jt in range(NJT):
                nc.tensor.matmul(
                    ps[:, 0:CS],
                    wo_sb[:, jt, mt * 128:mt * 128 + 128],
                    onT_sb[:, jt, qc * QC + cs2 * CS:qc * QC + (cs2 + 1) * CS],
                    start=(jt == 0),
                    stop=(jt == NJT - 1),
                )
            ob = oc_pool.tile([128, CS], BF16, tag="oc", name="oc")
            if copy_eng == "scalar":
                nc.scalar.copy(out=ob, in_=ps[:, 0:CS])
            else:
                nc.vector.tensor_copy(out=ob, in_=ps[:, 0:CS])
            (dma_eng or nc.sync).dma_start(
                out=outT[mt * 128:mt * 128 + 128,
                         qc * QC + cs2 * CS:qc * QC + (cs2 + 1) * CS],
                in_=ob,
            )

        # ---- emission schedule ------------------------------------------
        # DMA transfers serialize on the shared DMA engines (~1.2us per
        # 256KB chunk), so everything rides ONE hwdge queue in strict
        # deadline order: the critical prefix (wq, xq0, xq1, wk, xk0) gates
        # the first score at ~10us; the rest interleaves by fill-slot need.
        nc.sync.dma_start(out=wq_sb[0], in_=wqh.rearrange("(t p) j -> p t j", p=128))
        nc.sync.dma_start(out=wq_sb[1], in_=wql.rearrange("(t p) j -> p t j", p=128))
        nc.sync.dma_start(out=bq_col, in_=bq.rearrange("a (t p) -> p (a t)", p=128))

        xk = [None] * NXC
        xq = [None] * NXC
        xv = [None] * NXC
        xq[0] = load_x(nc.sync, xqh, xql, 0)
        xq[1] = load_x(nc.sync, xqh, xql, 1)
        nc.sync.dma_start(out=wk_sb[0], in_=wkh.rearrange("(t p) j -> p t j", p=128))
        nc.sync.dma_start(out=wk_sb[1], in_=wkl.rearrange("(t p) j -> p t j", p=128))
        nc.sync.dma_start(out=bk_col, in_=bk.rearrange("a (t p) -> p (a t)", p=128))
        xk[0] = load_x(nc.sync, xkh, xkl, 0)
        xv[0] = load_x(nc.sync, xvh, xvl, 0)
        nc.sync.dma_start(out=wv_sb[0], in_=wvh.rearrange("(t p) j -> p t j", p=128))
        nc.sync.dma_start(out=wv_sb[1], in_=wvl.rearrange("(t p) j -> p t j", p=128))
        nc.sync.dma_start(out=ident_sb, in_=ident)
        xk[1] = load_x(nc.sync, xkh, xkl, 1)
        xv[1] = load_x(nc.sync, xvh, xvl, 1)
        xq[2] = load_x(nc.sync, xqh, xql, 2)
        xk[2] = load_x(nc.sync, xkh, xkl, 2)
        xv[2] = load_x(nc.sync, xvh, xvl, 2)
        xq[3] = load_x(nc.sync, xqh, xql, 3)
        xk[3] = load_x(nc.sync, xkh, xkl, 3)
        xv[3] = load_x(nc.sync, xvh, xvl, 3)
        nc.sync.dma_start(out=wo_sb, in_=wo.rearrange("(t p) d -> p t d", p=128))

        # PE p-state warmup: a stream of throwaway matmuls keeps the PE busy
        # while the first x chunks stream in, so the real projections start
        # at full clock instead of pstate-low.
        warm_ps = psum.tile([128, CS], F32, tag="sp", name="warm")
        for _ in range(11):
            nc.tensor.matmul(warm_ps, warm_one, warm_row, start=True, stop=True,
                             skip_group_check=True)

        # pre-attention projections: phase A (qc0, pair0) only needs the
        # group-0 rows of qT (chunks 0,1) and kT (chunk 0); the group-1 and
        # later-chunk projections ride the fill slots.
        kqproj_g((wq_sb[0], wq_sb[1]), bq_col, qT_sb, xq[0], 0, 0, tag="acc0")
        kqproj_g((wq_sb[0], wq_sb[1]), bq_col, qT_sb, xq[1], 1, 0, tag="acc1")
        kqproj_g((wk_sb[0], wk_sb[1]), bk_col, kT_sb, xk[0], 0, 0, tag="acc2")

        # fill schedule: fills[phase][kt] = list of work items emitted after
        # that kt's scores/exps/attn@v.  Phase order is A=(qc0,p0),
        # B=(qc1,p0), C=(qc0,p1), D=(qc1,p1) so that B reuses group-0
        # kT/qT rows and group-1 projections spread into B/C.  Deadlines:
        # vproj(lt) in a slot <= lt; kproj (c,0) by A-slot 4c-1; qproj
        # (c2/c3, 0) by end of A (B reads qc1); kproj (*,1) + qproj
        # (c0/c1, 1) by end of B; qproj (c2/c3, 1) by end of C;
        # out-projection of qc0 fills D (needs C's epilogue).
        K, Q, V, O = "kproj", "qproj", "vproj", "outproj"
        slotsA = [[] for _ in range(NLT)]
        for lt in range(NLT):
            slotsA[lt].append((V, (lt // 4, lt)))
        for c in (1, 2, 3):
            slotsA[4 * (c - 1) + 2].append((K, (c, 0)))
        slotsA[5].append((Q, (2, 0)))
        slotsA[9].append((Q, (3, 0)))
        slotsB = [[] for _ in range(NLT)]
        slotsB[0].append((K, (0, 1)))
        slotsB[3].append((K, (1, 1)))
        slotsB[6].append((K, (2, 1)))
        slotsB[9].append((K, (3, 1)))
        slotsB[11].append((Q, (0, 1)))
        slotsB[13].append((Q, (1, 1)))
        slotsC = [[] for _ in range(NLT)]
        slotsC[1].append((Q, (2, 1)))
        slotsC[5].append((Q, (3, 1)))
        # outproj qc0 needs C's epilogue (lands ~kt0 of D): start at slot 2
        # and double up the last two slots.
        slotsD = [[] for _ in range(NLT)]
        d_order = list(range(2, 16)) + [14, 15]
        for i in range(16):
            slotsD[d_order[i]].append((O, (0, i // 2, i % 2)))
        fill_slots = [slotsA, slotsB, slotsC, slotsD]

        def run_fill(kind, args):
            if kind == K:
                c, g = args
                kqproj_g((wk_sb[0], wk_sb[1]), bk_col, kT_sb, xk[c], c, g, tag="sp")
            elif kind == Q:
                c, g = args
                kqproj_g((wq_sb[0], wq_sb[1]), bq_col, qT_sb, xq[c], c, g, tag="sp")
            elif kind == V:
                c, lt = args
                vproj_lt(xv[c], c, lt)
            else:
                outproj_chunk(*args)

        pending_epi = None
        for phase, (qc, pair) in enumerate([(0, 0), (1, 0), (0, 1), (1, 1)]):
            slots = fill_slots[phase]
            h0, h1 = 2 * pair, 2 * pair + 1
            accs = None
            prev = None
            for kt in range(NLT):
                st0 = score(h0, qc, kt)
                at0 = exp_(h0, st0)
                if pending_epi is not None:
                    # previous phase's epilogue rides behind this phase's
                    # first scores so they aren't queued after its transposes
                    epilogue(*pending_epi)
                    pending_epi = None
                if accs is None:
                    accs = make_accs()
                if prev is not None:
                    attnv(h0, kt - 1, prev[0], accs)
                st1 = score(h1, qc, kt)
                at1 = exp_(h1, st1)
                if prev is not None:
                    attnv(h1, kt - 1, prev[1], accs)
                prev = (at0, at1)
                for item in slots[kt]:
                    run_fill(*item)
            attnv(h0, NLT - 1, prev[0], accs)
            attnv(h1, NLT - 1, prev[1], accs)
            pending_epi = (pair, qc, accs)

        # tail: out-projection of qc1 - the first half rides inside the last
        # epilogue (right after the qt0-3 transposes land in onT)
        tags = ["st0", "st1", "acc0", "acc1"]

        def _tail_first_half():
            for i, mt in enumerate(range(NDT)):
                outproj_chunk(1, mt, 0, tag=tags[i % len(tags)],
                              copy_eng="scalar" if i % 2 else "vector",
                              dma_eng=nc.scalar if i % 2 else nc.sync)

        epilogue(*pending_epi, mid=_tail_first_half)
        for i, mt in enumerate(range(NDT)):
            outproj_chunk(1, mt, 1, tag=tags[i % len(tags)],
                          copy_eng="scalar" if i % 2 else "vector",
                          dma_eng=nc.scalar if i % 2 else nc.sync)

    nc.compile()
    return nc


_NC_CACHE = {}


def _get_nc():
    if "nc" not in _NC_CACHE:
        _NC_CACHE["nc"] = build_attention_core()
    return _NC_CACHE["nc"]


def _bf16(x):
    return np.asarray(x, np.float32).astype(ml_dtypes.bfloat16)


def _hilo(x):
    """fp8e4m3 hi+lo split (IEEE e4m3: bias 8, max 240)."""
    xh = np.asarray(x, np.float32).astype(ml_dtypes.float8_e4m3)
    xl = (np.asarray(x, np.float32) - xh.astype(np.float32)).astype(
        ml_dtypes.float8_e4m3)
    return np.ascontiguousarray(xh), np.ascontiguousarray(xl)


def shard_inputs(query, key_, value, Wq, bq, Wk, bk, Wv, bv, Wo, bo,
                 B=2, H=16, H_LOC=4, HD=64):
    """Host-side sharding: core c -> (batch c//4, head-group c%4)."""
    groups = H // H_LOC
    xT = [np.ascontiguousarray(np.asarray(x, np.float32).transpose(0, 2, 1))
          for x in (query, key_, value)]
    xT_hl = [[_hilo(xT[s][b]) for b in range(B)] for s in range(3)]
    Wq, Wk, Wv, Wo = (np.asarray(w, np.float32) for w in (Wq, Wk, Wv, Wo))
    bq, bk = (np.asarray(b, np.float32) for b in (bq, bk))
    ident = np.eye(128, dtype=np.float32).astype(ml_dtypes.bfloat16)
    in_maps = []
    for c in range(B * groups):
        b, g = divmod(c, groups)
        js = slice(g * H_LOC * HD, (g + 1) * H_LOC * HD)
        qh, ql = _hilo(Wq[:, js] * WBOOST)
        kh, kl = _hilo(Wk[:, js] * WBOOST)
        vh, vl = _hilo(Wv[:, js] * WBOOST)
        in_maps.append({
            "xqh": xT_hl[0][b][0], "xql": xT_hl[0][b][1],
            "xkh": xT_hl[1][b][0], "xkl": xT_hl[1][b][1],
            "xvh": xT_hl[2][b][0], "xvl": xT_hl[2][b][1],
            "wqh": qh, "wql": ql,
            "wkh": kh, "wkl": kl,
            "wvh": vh, "wvl": vl,
            "wo": _bf16(np.ascontiguousarray(Wo[js, :]) / WBOOST),
            "bq": np.ascontiguousarray(bq[None, js]) * WBOOST,
            "bk": np.ascontiguousarray(bk[None, js]) * WBOOST,
            "ident": ident,
        })
    return in_maps


def kernel(query, key_, value, Wq, bq, Wk, bk, Wv, bv, Wo, bo):
    B, L, D = 2, 2048, 1024
    groups = 4
    nc = _get_nc()
    in_maps = shard_inputs(query, key_, value, Wq, bq, Wk, bk, Wv, bv, Wo, bo)
    res = run_bass_kernel_spmd(nc, in_maps, list(range(8))).results
    out = np.empty((B, L, D), np.float32)
    # bv commutes through the normalized attention: out += (bv @ Wo + bo)
    bo_eff = (np.asarray(bv, np.float32) @ np.asarray(Wo, np.float32)
              + np.asarray(bo, np.float32))
    for b in range(B):
        acc = res[b * groups]["outT"].astype(np.float32)
        for g in range(1, groups):
            acc = acc + res[b * groups + g]["outT"]
        out[b] = acc.T + bo_eff
    return out


# revision 15
# speedup vs baseline: 1.1017x; 1.0227x over previous
"""Memory-efficient multi-head attention on 8 TRN2 NeuronCores.

Problem (hardcoded): B=2, L=2048, D=1024, H=16, HD=64.
  out = softmax((X_q Wq + bq)(X_k Wk + bk)^T / sqrt(HD)) (X_v Wv + bv) Wo + bo

Sharding: 8 cores = 2 batches x 4 head-groups (4 heads each).  Each core gets
its batch's activations (transposed), plus its head-group's weight slices, and
produces a partial pre-bias output out^T [D, L] (bf16).  Host sums the 4
partials per batch and adds bo_eff = bo + bv @ Wo (bv commutes through the
softmax-normalized attention, bk cancels in softmax entirely but is still
applied for exactness of the q/k biases).

v3 design vs the bf16 v2 baseline (190.9us):
  - q/k/v projections run as fp8 hi+lo DoubleRow matmuls: W is host-scaled
    by 32 (so the lo residuals stay in e4m3 normal range), split into
    Wh=fp8(32W), Wl=fp8(32W-Wh), and likewise X into Xh/Xl.  Each dt-pair
    contributes three DoubleRow matmuls (XhWh+XlWl fused diagonally is NOT
    used; terms are Wh*Xh, Wh*Xl, Wl*Xh across dt-pair subtiles), i.e.
    12 matmuls x 256 cyc per (chunk, group) instead of 8 x 512: -25% PE.
    Dropping the Wl*Xl term keeps bf16-level accuracy (ablated 6.7e-3 rel
    vs 8.1e-3 for all-bf16).  The x32 scale folds out exactly: exp scale
    *= 2^-10, host Wo *= 2^-5.
  - ScalarE runs ONLY the exp stream (128 activations of [128,1024]):
    normalization muls and PSUM->SBUF copies all move to DVE; tail copies
    (after the last exp) use ScalarE.  The exp stream is the 133us wall
    the schedule is built around.
  - Scores / attn@v / out-projection stay bf16 (fp8 ablates to >2e-2 rel:
    concentrated-attention rows do not average quantization noise away).
  - ident / onp / tps are bf16 (transpose cost 1.0 cyc/row vs 1.5 f32r).
  - Fill schedule rebalanced so phases B/C/D also carry projection /
    out-projection work and the PE never idles behind the exp stream.
"""

import numpy as np
import ml_dtypes

import concourse.mybir as mybir
import concourse.tile as tile
from concourse import bacc
from concourse.bass_utils import run_bass_kernel_spmd

F32 = mybir.dt.float32
BF16 = mybir.dt.bfloat16
F8 = mybir.dt.float8e4
DR = mybir.MatmulPerfMode.DoubleRow

WBOOST = 32.0  # host W scale; folds out via exp scale and Wo scale


def build_attention_core(L=2048, D=1024, H_LOC=4, HD=64, debug=False):
    JC = H_LOC * HD                   # local head channels (256)
    NJT = JC // 128                   # j-tiles (2)
    NDT = D // 128                    # contraction tiles (8)
    NPR = NDT // 2                    # dt-pairs for DoubleRow (4)
    NLT = L // 128                    # k-position tiles (16)
    XC = 512                          # x chunk width
    NXC = L // XC                     # x chunks per stream (4)
    QC = 1024                         # attention q chunk
    NQC = L // QC                     # 2
    NQT = QC // 128                   # q tiles per chunk (8)
    CS = 512                          # matmul N chunk
    # exp scale: 1/sqrt(HD) with the host-side W x32 boost (q,k each x32)
    SCALE = float(1.0 / (np.sqrt(HD) * WBOOST * WBOOST))

    nc = bacc.Bacc("TRN2", target_bir_lowering=False, debug=False, num_devices=8)

    xqh = nc.dram_tensor("xqh", [D, L], F8, kind="ExternalInput").ap()
    xql = nc.dram_tensor("xql", [D, L], F8, kind="ExternalInput").ap()
    xkh = nc.dram_tensor("xkh", [D, L], F8, kind="ExternalInput").ap()
    xkl = nc.dram_tensor("xkl", [D, L], F8, kind="ExternalInput").ap()
    xvh = nc.dram_tensor("xvh", [D, L], F8, kind="ExternalInput").ap()
    xvl = nc.dram_tensor("xvl", [D, L], F8, kind="ExternalInput").ap()
    # weights are HOST-PACKED partition-major ([128, NDT*JC], one contiguous
    # 2KB descriptor per partition): fp8 [D, JC] row-major would emit 256B
    # descriptors, which the DMA bus charges 2x for.
    wqh = nc.dram_tensor("wqh", [128, NDT * JC], F8, kind="ExternalInput").ap()
    wql = nc.dram_tensor("wql", [128, NDT * JC], F8, kind="ExternalInput").ap()
    wkh = nc.dram_tensor("wkh", [128, NDT * JC], F8, kind="ExternalInput").ap()
    wkl = nc.dram_tensor("wkl", [128, NDT * JC], F8, kind="ExternalInput").ap()
    wvh = nc.dram_tensor("wvh", [128, NDT * JC], F8, kind="ExternalInput").ap()
    wvl = nc.dram_tensor("wvl", [128, NDT * JC], F8, kind="ExternalInput").ap()
    wo = nc.dram_tensor("wo", [JC, D], BF16, kind="ExternalInput").ap()
    bq = nc.dram_tensor("bq", [1, JC], F32, kind="ExternalInput").ap()
    bk = nc.dram_tensor("bk", [1, JC], F32, kind="ExternalInput").ap()
    ident = nc.dram_tensor("ident", [128, 128], BF16, kind="ExternalInput").ap()
    outT = nc.dram_tensor("outT", [D, L], BF16, kind="ExternalOutput").ap()

    from contextlib import ExitStack

    with (
        tile.TileContext(nc) as tc,
        ExitStack() as stack,
        nc.allow_low_precision(reason="bf16/fp8-hilo operands; psum accum is f32"),
    ):
        consts = stack.enter_context(tc.tile_pool(name="consts", bufs=1))
        wpool = stack.enter_context(tc.tile_pool(name="wpool", bufs=1))
        prod = stack.enter_context(tc.tile_pool(name="prod", bufs=1))
        xt_pool = stack.enter_context(tc.tile_pool(name="xt", bufs=24))
        att_pool = stack.enter_context(tc.tile_pool(name="att", bufs=6))
        onp_pool = stack.enter_context(tc.tile_pool(name="onp", bufs=2))
        rd_pool = stack.enter_context(tc.tile_pool(name="rd", bufs=2))
        oc_pool = stack.enter_context(tc.tile_pool(name="oc", bufs=8))
        # PSUM banks: st0/st1 [128,1024] (2 each) + acc0/acc1/acc2 (1 each,
        # 65-wide attn@v regions packed 7 per bank) + sp [128,512] (1) = 8.
        psum = stack.enter_context(tc.tile_pool(name="psum", bufs=1, space="PSUM"))

        warm_row = consts.tile([1, CS], BF16)    # PE p-state warmup stream
        nc.vector.memset(warm_row, 0.0)
        warm_one = consts.tile([1, 128], BF16)
        nc.vector.memset(warm_one, 0.0)
        vones = consts.tile([128, NLT * H_LOC], BF16)
        nc.vector.memset(vones, 1.0)

        wq_sb = [wpool.tile([128, NDT, JC], F8, tag=f"wq{i}", name="wq")
                 for i in range(2)]
        wk_sb = [wpool.tile([128, NDT, JC], F8, tag=f"wk{i}", name="wk")
                 for i in range(2)]
        wv_sb = [wpool.tile([128, NDT, JC], F8, tag=f"wv{i}", name="wv")
                 for i in range(2)]
        wo_sb = wpool.tile([128, NJT, D], BF16, tag="wo")
        bq_col = wpool.tile([128, NJT], F32, tag="bq")
        bk_col = wpool.tile([128, NJT], F32, tag="bk")
        ident_sb = wpool.tile([128, 128], BF16, tag="ident")

        kT_sb = prod.tile([128, NJT, L], BF16, tag="kT")   # [j%128, jt, kpos]
        qT_sb = prod.tile([128, NJT, L], BF16, tag="qT")
        v_sb = prod.tile([128, NLT, H_LOC, HD + 1], BF16, tag="v")  # [kpos%128, kt, h, ch|1]
        onT_sb = prod.tile([128, NJT, L], BF16, tag="onT")  # [j%128, jt, qpos]
        nc.vector.tensor_copy(
            out=v_sb[:, :, :, HD:HD + 1].rearrange("p a h c -> p (a h c)"),
            in_=vones,
        )

        # ---- emission helpers -------------------------------------------
        def load_x(eng, x_h, x_l, c):
            """Two-DMA chunk load: x^T hi/lo [:, c*XC:(c+1)*XC] -> [128, NDT, XC]."""
            ts = []
            for x_dram in (x_h, x_l):
                x_t = xt_pool.tile([128, NDT, XC], F8, tag="xt", name="xt")
                eng.dma_start(
                    out=x_t,
                    in_=x_dram.rearrange("(t p) l -> p t l", p=128)[
                        :, :, c * XC:(c + 1) * XC],
                )
                ts.append(x_t)
            return ts

        def kqproj_g(w_hl, b_col, dst, x_hl, c, g, tag=None):
            """dst[j, l] = sum_d W[d, j] x^T[d, l] + b[j] for one (chunk, group).
            fp8 hi-lo DoubleRow over dt-pairs: Wh*Xh + Wh*Xl + Wl*Xh."""
            tag = tag or "sp"
            shape = [128, QC] if tag.startswith("st") else [128, CS]
            ps = psum.tile(shape, F32, tag=tag, name="pj")
            w_h, w_l = w_hl
            x_h, x_l = x_hl
            js = slice(g * 128, (g + 1) * 128)
            for t in range(NPR):
                sl = slice(2 * t, 2 * t + 2)
                nc.tensor.matmul(ps[:, 0:XC], w_h[:, sl, js], x_h[:, sl, :],
                                 start=(t == 0), stop=False, perf_mode=DR)
                nc.tensor.matmul(ps[:, 0:XC], w_h[:, sl, js], x_l[:, sl, :],
                                 start=False, stop=False, perf_mode=DR)
                nc.tensor.matmul(ps[:, 0:XC], w_l[:, sl, js], x_h[:, sl, :],
                                 start=False, stop=(t == NPR - 1), perf_mode=DR)
            nc.vector.tensor_scalar_add(
                dst[:, g, c * XC:(c + 1) * XC], ps[:, 0:XC], b_col[:, g:g + 1],
            )

        def vproj_lt(x_hl, c, lt):
            """v[kpos, j] = sum_d x^T[d, kpos] W[d, j], one 128-row tile.
            fp8 hi-lo DoubleRow; bv is folded into bo on the host."""
            ps = psum.tile([128, CS], F32, tag="sp", name="pv")
            x_h, x_l = x_hl
            off = lt * 128 - c * XC
            ks = slice(off, off + 128)
            for t in range(NPR):
                sl = slice(2 * t, 2 * t + 2)
                nc.tensor.matmul(ps[:, 0:JC], x_h[:, sl, ks], wv_sb[0][:, sl, :],
                                 start=(t == 0), stop=False, perf_mode=DR)
                nc.tensor.matmul(ps[:, 0:JC], x_h[:, sl, ks], wv_sb[1][:, sl, :],
                                 start=False, stop=False, perf_mode=DR)
                nc.tensor.matmul(ps[:, 0:JC], x_l[:, sl, ks], wv_sb[0][:, sl, :],
                                 start=False, stop=(t == NPR - 1), perf_mode=DR)
            nc.vector.tensor_copy(
                out=v_sb[:, lt, :, 0:HD],
                in_=ps[:, 0:JC].rearrange("p (h c) -> p h c", h=H_LOC),
            )

        def score(h, qc, kt):
            jt, hp = h // 2, (h % 2) * HD
            st = psum.tile([128, QC], F32, tag=f"st{h % 2}", name=f"st{h % 2}")
            for cs in range(QC // CS):
                nc.tensor.matmul(
                    st[:, cs * CS:(cs + 1) * CS],
                    kT_sb[hp:hp + HD, jt, kt * 128:kt * 128 + 128],
                    qT_sb[hp:hp + HD, jt, qc * QC + cs * CS:qc * QC + (cs + 1) * CS],
                    start=True,
                    stop=True,
                )
            return st

        def exp_(h, st):
            at = att_pool.tile([128, QC], BF16, tag=f"at{h % 2}", name=f"at{h % 2}")
            nc.scalar.activation(
                out=at, in_=st, func=mybir.ActivationFunctionType.Exp, scale=SCALE,
            )
            return at

        # acc region map: r = h2*8 + qt -> psum tile r//7, 65-col slot r%7.
        # (65-wide regions packed 7 per 2KB bank so none straddles a bank.)
        ACC_FIRST = (0, 7, 14)
        ACC_LAST = (6, 13, 15)

        def attnv(h, kt, at, accs):
            # HW semantics: a start=True matmul zeroes the ENTIRE psum bank,
            # so exactly one start per bank (first write) and one stop (last).
            for qt in range(NQT):
                r = (h % 2) * NQT + qt
                t, slot = divmod(r, 7)
                nc.tensor.matmul(
                    accs[t][:, slot * 65:slot * 65 + 65],
                    at[:, qt * 128:(qt + 1) * 128],
                    v_sb[:, kt, h, :],
                    start=(kt == 0 and r == ACC_FIRST[t]),
                    stop=(kt == NLT - 1 and r == ACC_LAST[t]),
                    skip_group_check=True,
                )

        def epilogue(pair, qc, accs, act_muls=False, mid=None):
            """onorm_pair[q, qt, h2*64+ch] = acc * recip(den-col); PE-transpose
            each [128,128] q x (h2,ch) tile into onT[j, qpos].  act_muls=True
            (tail only, after the last exp) splits the muls with ScalarE."""
            rdens = []
            for t, n in ((0, 7), (1, 7), (2, 2)):
                rd = rd_pool.tile([128, n], F32, tag=f"rd{t}", name="rd")
                nc.vector.reciprocal(out=rd, in_=accs[t][:, 64::65][:, 0:n])
                rdens.append(rd)
            onp = onp_pool.tile([128, NQT, 128], BF16, tag="onp", name="onp")
            tps = [psum.tile([128, NQT // 2, 128], BF16, tag=t, name="tp")
                   for t in ("sp", "acc2")]
            for qt in range(NQT):
                for h2 in range(2):
                    r = h2 * NQT + qt
                    t, slot = divmod(r, 7)
                    if act_muls and h2 == 1:
                        nc.scalar.activation(
                            out=onp[:, qt, h2 * HD:(h2 + 1) * HD],
                            in_=accs[t][:, slot * 65:slot * 65 + 64],
                            func=mybir.ActivationFunctionType.Copy,
                            scale=rdens[t][:, slot:slot + 1],
                        )
                    else:
                        nc.vector.tensor_scalar_mul(
                            onp[:, qt, h2 * HD:(h2 + 1) * HD],
                            accs[t][:, slot * 65:slot * 65 + 64],
                            rdens[t][:, slot:slot + 1],
                        )
                half, q4 = divmod(qt, NQT // 2)
                nc.tensor.matmul(
                    tps[half][:, q4, :], onp[:, qt, :], ident_sb,
                    is_transpose=True, start=(q4 == 0), stop=(q4 == NQT // 2 - 1),
                    skip_group_check=True,
                )
                if q4 == NQT // 2 - 1:
                    nc.vector.tensor_copy(
                        out=onT_sb[:, pair,
                                   qc * QC + half * (QC // 2):
                                   qc * QC + (half + 1) * (QC // 2)],
                        in_=tps[half].rearrange("p a b -> p (a b)"),
                    )
                    if mid is not None and half == 0:
                        mid()

        def make_accs():
            return [psum.tile([128, w], F32, tag=f"acc{i}", name=f"acc{i}")
                    for i, w in ((0, 455), (1, 455), (2, 130))]

        def outproj_chunk(qc, mt, cs2, tag="sp", copy_eng="vector",
                          dma_eng=None):
            shape = [128, QC] if tag.startswith("st") else [128, CS]
            ps = psum.tile(shape, F32, tag=tag, name="po")
            for jt in range(NJT):
                nc.tensor.matmul(
                    ps[:, 0:CS],
                    wo_sb[:, jt, mt * 128:mt * 128 + 128],
                    onT_sb[:, jt, qc * QC + cs2 * CS:qc * QC + (cs2 + 1) * CS],
                    start=(jt == 0),
                    stop=(jt == NJT - 1),
                )
            ob = oc_pool.tile([128, CS], BF16, tag="oc", name="oc")
            if copy_eng == "scalar":
                nc.scalar.copy(out=ob, in_=ps[:, 0:CS])
            else:
                nc.vector.tensor_copy(out=ob, in_=ps[:, 0:CS])
            (dma_eng or nc.sync).dma_start(
                out=outT[mt * 128:mt * 128 + 128,
                         qc * QC + cs2 * CS:qc * QC + (cs2 + 1) * CS],
                in_=ob,
            )

        # ---- emission schedule ------------------------------------------
        # DMA transfers serialize on the shared DMA engines (~1.2us per
        # 256KB chunk), so everything rides ONE hwdge queue in strict
        # deadline order: the critical prefix (wq, xq0, xq1, wk, xk0) gates
        # the first score at ~10us; the rest interleaves by fill-slot need.
        nc.sync.dma_start(out=wq_sb[0], in_=wqh.rearrange("p (t j) -> p t j", t=8))
        nc.sync.dma_start(out=wq_sb[1], in_=wql.rearrange("p (t j) -> p t j", t=8))
        nc.sync.dma_start(out=bq_col, in_=bq.rearrange("a (t p) -> p (a t)", p=128))

        xk = [None] * NXC
        xq = [None] * NXC
        xv = [None] * NXC
        xq[0] = load_x(nc.sync, xqh, xql, 0)
        xq[1] = load_x(nc.sync, xqh, xql, 1)
        nc.sync.dma_start(out=wk_sb[0], in_=wkh.rearrange("p (t j) -> p t j", t=8))
        nc.sync.dma_start(out=wk_sb[1], in_=wkl.rearrange("p (t j) -> p t j", t=8))
        nc.sync.dma_start(out=bk_col, in_=bk.rearrange("a (t p) -> p (a t)", p=128))
        xk[0] = load_x(nc.sync, xkh, xkl, 0)
        xv[0] = load_x(nc.sync, xvh, xvl, 0)
        nc.sync.dma_start(out=wv_sb[0], in_=wvh.rearrange("p (t j) -> p t j", t=8))
        nc.sync.dma_start(out=wv_sb[1], in_=wvl.rearrange("p (t j) -> p t j", t=8))
        nc.sync.dma_start(out=ident_sb, in_=ident)
        xk[1] = load_x(nc.sync, xkh, xkl, 1)
        xv[1] = load_x(nc.sync, xvh, xvl, 1)
        xq[2] = load_x(nc.sync, xqh, xql, 2)
        xk[2] = load_x(nc.sync, xkh, xkl, 2)
        xv[2] = load_x(nc.sync, xvh, xvl, 2)
        xq[3] = load_x(nc.sync, xqh, xql, 3)
        xk[3] = load_x(nc.sync, xkh, xkl, 3)
        xv[3] = load_x(nc.sync, xvh, xvl, 3)
        nc.sync.dma_start(out=wo_sb, in_=wo.rearrange("(t p) d -> p t d", p=128))

        # PE p-state warmup: a stream of throwaway matmuls keeps the PE busy
        # while the first x chunks stream in, so the real projections start
        # at full clock instead of pstate-low.
        warm_ps = psum.tile([128, CS], F32, tag="sp", name="warm")
        for _ in range(11):
            nc.tensor.matmul(warm_ps, warm_one, warm_row, start=True, stop=True,
                             skip_group_check=True)

        # pre-attention projections: phase A (qc0, pair0) only needs the
        # group-0 rows of qT (chunks 0,1) and kT (chunk 0); the group-1 and
        # later-chunk projections ride the fill slots.
        kqproj_g((wq_sb[0], wq_sb[1]), bq_col, qT_sb, xq[0], 0, 0, tag="acc0")
        kqproj_g((wq_sb[0], wq_sb[1]), bq_col, qT_sb, xq[1], 1, 0, tag="acc1")
        kqproj_g((wk_sb[0], wk_sb[1]), bk_col, kT_sb, xk[0], 0, 0, tag="acc2")

        # fill schedule: fills[phase][kt] = list of work items emitted after
        # that kt's scores/exps/attn@v.  Phase order is A=(qc0,p0),
        # B=(qc1,p0), C=(qc0,p1), D=(qc1,p1) so that B reuses group-0
        # kT/qT rows and group-1 projections spread into B/C.  Deadlines:
        # vproj(lt) in a slot <= lt; kproj (c,0) by A-slot 4c-1; qproj
        # (c2/c3, 0) by end of A (B reads qc1); kproj (*,1) + qproj
        # (c0/c1, 1) by end of B; qproj (c2/c3, 1) by end of C;
        # out-projection of qc0 fills D (needs C's epilogue).
        K, Q, V, O = "kproj", "qproj", "vproj", "outproj"
        slotsA = [[] for _ in range(NLT)]
        for lt in range(NLT):
            slotsA[lt].append((V, (lt // 4, lt)))
        for c in (1, 2, 3):
            slotsA[4 * (c - 1) + 2].append((K, (c, 0)))
        slotsA[5].append((Q, (2, 0)))
        slotsA[9].append((Q, (3, 0)))
        slotsB = [[] for _ in range(NLT)]
        slotsB[0].append((K, (0, 1)))
        slotsB[3].append((K, (1, 1)))
        slotsB[6].append((K, (2, 1)))
        slotsB[9].append((K, (3, 1)))
        slotsB[11].append((Q, (0, 1)))
        slotsB[13].append((Q, (1, 1)))
        slotsC = [[] for _ in range(NLT)]
        slotsC[1].append((Q, (2, 1)))
        slotsC[5].append((Q, (3, 1)))
        # outproj qc0 needs C's epilogue (lands ~kt0 of D): start at slot 2
        # and double up the last two slots.
        slotsD = [[] for _ in range(NLT)]
        d_order = list(range(2, 16)) + [14, 15]
        for i in range(16):
            slotsD[d_order[i]].append((O, (0, i // 2, i % 2)))
        fill_slots = [slotsA, slotsB, slotsC, slotsD]

        def run_fill(kind, args):
            if kind == K:
                c, g = args
                kqproj_g((wk_sb[0], wk_sb[1]), bk_col, kT_sb, xk[c], c, g, tag="sp")
            elif kind == Q:
                c, g = args
                kqproj_g((wq_sb[0], wq_sb[1]), bq_col, qT_sb, xq[c], c, g, tag="sp")
            elif kind == V:
                c, lt = args
                vproj_lt(xv[c], c, lt)
            else:
                outproj_chunk(*args)

        pending_epi = None
        for phase, (qc, pair) in enumerate([(0, 0), (1, 0), (0, 1), (1, 1)]):
            slots = fill_slots[phase]
            h0, h1 = 2 * pair, 2 * pair + 1
            accs = None
            prev = None
            for kt in range(NLT):
                st0 = score(h0, qc, kt)
                at0 = exp_(h0, st0)
                if pending_epi is not None:
                    # previous phase's epilogue rides behind this phase's
                    # first scores so they aren't queued after its transposes
                    epilogue(*pending_epi)
                    pending_epi = None
                if accs is None:
                    accs = make_accs()
                if prev is not None:
                    attnv(h0, kt - 1, prev[0], accs)
                st1 = score(h1, qc, kt)
                at1 = exp_(h1, st1)
                if prev is not None:
                    attnv(h1, kt - 1, prev[1], accs)
                prev = (at0, at1)
                for item in slots[kt]:
                    run_fill(*item)
            attnv(h0, NLT - 1, prev[0], accs)
            attnv(h1, NLT - 1, prev[1], accs)
            pending_epi = (pair, qc, accs)

        # tail: out-projection of qc1 in full-QC-wide chunks on the st psum
        # banks (free after the last exp).  The first half rides inside the
        # last epilogue (right after the qt0-3 transposes land in onT).
        def outproj_wide(mt, i):
            # matmul outputs must not cross a 2KB psum bank: 512-col slices
            ps = psum.tile([128, QC], F32, tag=f"st{i % 2}", name="po")
            for cs in range(QC // CS):
                for jt in range(NJT):
                    nc.tensor.matmul(
                        ps[:, cs * CS:(cs + 1) * CS],
                        wo_sb[:, jt, mt * 128:mt * 128 + 128],
                        onT_sb[:, jt, QC + cs * CS:QC + (cs + 1) * CS],
                        start=(jt == 0), stop=(jt == NJT - 1),
                    )
            ob = oc_pool.tile([128, QC], BF16, tag="ocw", bufs=4, name="ocw")
            if i % 2:
                nc.scalar.copy(out=ob, in_=ps)
            else:
                nc.vector.tensor_copy(out=ob, in_=ps)
            (nc.scalar if i % 2 else nc.sync).dma_start(
                out=outT[mt * 128:mt * 128 + 128, QC:2 * QC], in_=ob,
            )

        def _tail_first_half():
            for i, mt in enumerate(range(4)):
                outproj_wide(mt, i)

        epilogue(*pending_epi, act_muls=True, mid=_tail_first_half)
        for i, mt in enumerate(range(4, NDT)):
            outproj_wide(mt, i)

    nc.compile()
    return nc


_NC_CACHE = {}


def _get_nc():
    if "nc" not in _NC_CACHE:
        _NC_CACHE["nc"] = build_attention_core()
    return _NC_CACHE["nc"]


def _bf16(x):
    return np.asarray(x, np.float32).astype(ml_dtypes.bfloat16)


def _hilo(x):
    """fp8e4m3 hi+lo split (IEEE e4m3: bias 8, max 240)."""
    xh = np.asarray(x, np.float32).astype(ml_dtypes.float8_e4m3)
    xl = (np.asarray(x, np.float32) - xh.astype(np.float32)).astype(
        ml_dtypes.float8_e4m3)
    return np.ascontiguousarray(xh), np.ascontiguousarray(xl)


def _pack_w(w):
    """[D, JC] -> [128, NDT*JC] partition-major (one 2KB DMA descriptor per
    partition instead of 8x 256B ones)."""
    d, jc = w.shape
    return np.ascontiguousarray(
        w.reshape(d // 128, 128, jc).transpose(1, 0, 2).reshape(128, -1))


def shard_inputs(query, key_, value, Wq, bq, Wk, bk, Wv, bv, Wo, bo,
                 B=2, H=16, H_LOC=4, HD=64):
    """Host-side sharding: core c -> (batch c//4, head-group c%4)."""
    groups = H // H_LOC
    xT = [np.ascontiguousarray(np.asarray(x, np.float32).transpose(0, 2, 1))
          for x in (query, key_, value)]
    xT_hl = [[_hilo(xT[s][b]) for b in range(B)] for s in range(3)]
    Wq, Wk, Wv, Wo = (np.asarray(w, np.float32) for w in (Wq, Wk, Wv, Wo))
    bq, bk = (np.asarray(b, np.float32) for b in (bq, bk))
    ident = np.eye(128, dtype=np.float32).astype(ml_dtypes.bfloat16)
    in_maps = []
    for c in range(B * groups):
        b, g = divmod(c, groups)
        js = slice(g * H_LOC * HD, (g + 1) * H_LOC * HD)
        qh, ql = _hilo(Wq[:, js] * WBOOST)
        kh, kl = _hilo(Wk[:, js] * WBOOST)
        vh, vl = _hilo(Wv[:, js] * WBOOST)
        in_maps.append({
            "xqh": xT_hl[0][b][0], "xql": xT_hl[0][b][1],
            "xkh": xT_hl[1][b][0], "xkl": xT_hl[1][b][1],
            "xvh": xT_hl[2][b][0], "xvl": xT_hl[2][b][1],
            "wqh": _pack_w(qh), "wql": _pack_w(ql),
            "wkh": _pack_w(kh), "wkl": _pack_w(kl),
            "wvh": _pack_w(vh), "wvl": _pack_w(vl),
            "wo": _bf16(np.ascontiguousarray(Wo[js, :]) / WBOOST),
            "bq": np.ascontiguousarray(bq[None, js]) * WBOOST,
            "bk": np.ascontiguousarray(bk[None, js]) * WBOOST,
            "ident": ident,
        })
    return in_maps


def kernel(query, key_, value, Wq, bq, Wk, bk, Wv, bv, Wo, bo):
    B, L, D = 2, 2048, 1024
    groups = 4
    nc = _get_nc()
    in_maps = shard_inputs(query, key_, value, Wq, bq, Wk, bk, Wv, bv, Wo, bo)
    res = run_bass_kernel_spmd(nc, in_maps, list(range(8))).results
    out = np.empty((B, L, D), np.float32)
    # bv commutes through the normalized attention: out += (bv @ Wo + bo)
    bo_eff = (np.asarray(bv, np.float32) @ np.asarray(Wo, np.float32)
              + np.asarray(bo, np.float32))
    for b in range(B):
        acc = res[b * groups]["outT"].astype(np.float32)
        for g in range(1, groups):
            acc = acc + res[b * groups + g]["outT"]
        out[b] = acc.T + bo_eff
    return out
